# revision 23
# baseline (speedup 1.0000x reference)
"""CPI_DGLLife kernel for 8 Trainium2 NeuronCores (SPMD).

GCN over a 65536-node graph + protein conv1d branch + CPI head.
Sharding: data-parallel over the 512-graph batch (64 graphs / core).

v2 design:
- Phase-separated schedule: all edge gathers issue up-front on the gpsimd
  SWDGE queues (transpose-mode, bf16, 256B tokens); the protein conv runs
  dense on tensor/scalar/vector with no cross-engine blocking; reductions
  + GNN matmul chain run at the end.
- bf16 matmuls everywhere (1 cyc/row at any free-dim size).
- rsqrt(deg_out) folded into the gather tables; the two readout linears
  folded into one matmul (no activation between them); conv biases folded
  into the matmuls via a ones-row in the rhs (layers 1, 2, 4).
- 2-table split (32767/32767/2) + per-core lexicographic degree bundling
  cuts gather token padding from 2.0x to ~1.2x.
"""
import sys
sys.path.insert(0, "/opt/trn_rl_repo")
import contextlib
import numpy as np
import ml_dtypes

import concourse.bass as bass
import concourse.bacc as bacc
import concourse.tile as tile
from concourse import mybir
from concourse.bass_utils import run_bass_kernel_spmd
from concourse.masks import make_identity

bf16 = ml_dtypes.bfloat16
dt = mybir.dt
AF = mybir.ActivationFunctionType
ALU = mybir.AluOpType
AX = mybir.AxisListType

P = 128
N, E, B, L = 65536, 262144, 512, 1000
IN_DIM, HID, VOCAB = 74, 128, 25
CHANNELS = [HID, 96, 128, IN_DIM, HID]
NCORES = 8
GPC = B // NCORES              # graphs per core = 64
PPC = GPC                      # proteins per core = 64
TBASES = [0, 32767, 65534]
TNN = [32767, 32767, 2]
TOKCAP = 4096                  # max tokens per dma_gather instruction
NQ = 4


# ------------------------------------------------------------------ host prep
def _host_prep(inputs):
    graph_ids = np.asarray(inputs["graph_ids"])
    src = np.concatenate([np.asarray(inputs["edge_src"]).astype(np.int64),
                          np.arange(N, dtype=np.int64)])
    dst = np.concatenate([np.asarray(inputs["edge_dst"]).astype(np.int64),
                          np.arange(N, dtype=np.int64)])
    deg_out = np.bincount(src, minlength=N).astype(np.float32)
    deg_in = np.bincount(dst, minlength=N).astype(np.float32)
    NE = len(src)

    # gather tables: bf16 [rows, 128], row v+1 = X[v] * rsqrt(deg_out[v])
    nf = np.asarray(inputs["node_feats"], np.float32)
    nfs = nf * (1.0 / np.sqrt(deg_out))[:, None]
    tabs = []
    for T in range(3):
        tb = np.zeros((TNN[T] + 1, P), np.float32)
        tb[1:1 + TNN[T], :IN_DIM] = nfs[TBASES[T]:TBASES[T] + TNN[T]]
        tabs.append(tb.astype(bf16))

    tbl_of = np.digitize(src, TBASES[1:])          # table of each edge's src
    loc_of = (src - np.asarray(TBASES)[tbl_of] + 1).astype(np.int64)

    # per-dst-node per-table edge counts
    cnt = np.zeros((N, 3), np.int64)
    np.add.at(cnt, (dst, tbl_of), 1)

    core_node_lo = np.searchsorted(graph_ids, np.arange(0, B + 1, GPC))
    ncore_nodes = core_node_lo[1:] - core_node_lo[:-1]
    NT = int(np.ceil(ncore_nodes.max() / P))
    NPAD = NT * P

    # per-core node permutation: lexicographic descending by (c1, c0)
    perm = np.full((NCORES, NPAD), -1, np.int64)
    for c in range(NCORES):
        lo, hi = int(core_node_lo[c]), int(core_node_lo[c + 1])
        cc = cnt[lo:hi]
        order = np.lexsort((-cc[:, 0], -cc[:, 1])) + lo
        perm[c, :hi - lo] = order

    # k per (tile, table): max over cores and lanes (shared SPMD schedule)
    G = 128
    NG = P // G
    cnt_perm = np.zeros((NCORES, NPAD, 3), np.int64)
    m = perm >= 0
    cnt_perm[m] = cnt[perm[m]]
    kg = cnt_perm.reshape(NCORES, NT, NG, G, 3).max(axis=3).max(axis=0)

    # token stream offsets per (table, tile, group); instruction packing.
    # instructions are disjoint [off, off+ntok) ranges, each %128 tokens
    # (padding tokens point at table row 0 = zeros).
    tok_off = np.full((3, NT, NG), -1, np.int64)
    tok_total = [0, 0, 0]
    sched = []  # (T, off, ntok)
    for T in range(3):
        off = 0
        cur_off = 0
        for t in range(NT):
            blk = int(kg[t, :, T].sum()) * G
            if blk == 0:
                continue
            if off - cur_off + blk > TOKCAP and off > cur_off:
                off = int(np.ceil(off / 128)) * 128
                sched.append((T, cur_off, off - cur_off))
                cur_off = off
            for g in range(NG):
                if kg[t, g, T] > 0:
                    tok_off[T, t, g] = off
                    off += int(kg[t, g, T]) * G
        if off > cur_off:
            off = int(np.ceil(off / 128)) * 128
            sched.append((T, cur_off, off - cur_off))
        tok_total[T] = max(off, 128)

    # node -> (core, padded position)
    pos_of = np.full(N, -1, np.int64)
    core_of = np.full(N, -1, np.int64)
    for c in range(NCORES):
        pm = perm[c]
        v = pm >= 0
        pos_of[pm[v]] = np.arange(NPAD)[v]
        core_of[pm[v]] = c

    # slot of each edge within its (core, tile, lane, table) group
    ec = core_of[dst]
    et = pos_of[dst] // P
    ep = pos_of[dst] % P
    key = (((ec * NT + et) * P + ep) * 3 + tbl_of)
    order = np.argsort(key, kind="stable")
    ks = key[order]
    starts = np.r_[0, np.flatnonzero(np.diff(ks)) + 1]
    grp_len = np.diff(np.r_[starts, NE])
    slot_sorted = np.arange(NE) - np.repeat(starts, grp_len)
    slot = np.empty(NE, np.int64)
    slot[order] = slot_sorted

    # token position (non-transpose layout): off(tile,T) + slot*128 + lane
    tok_pos = tok_off[tbl_of, et, 0] + slot * P + ep
    idx_flat = [np.zeros((NCORES, tok_total[T]), np.int16) for T in range(3)]
    for T in range(3):
        mT = tbl_of == T
        idx_flat[T][ec[mT], tok_pos[mT]] = loc_of[mT].astype(np.int16)

    def wrap(a):  # token-major -> wrapped [128, tokens//16]
        ncol = a.shape[1] // 16
        w = a.reshape(a.shape[0], ncol, 16).transpose(0, 2, 1)
        return np.ascontiguousarray(np.tile(w, (1, 8, 1)))

    idx_wrapped = [wrap(ix) for ix in idx_flat]

    # rsqrt(deg_in) per permuted lane, laid out [P, NT]
    rdgi = np.ones((NCORES, NPAD), np.float32)
    rdgi[m] = 1.0 / np.sqrt(deg_in[perm[m]])
    rdgi_pt = np.ascontiguousarray(
        rdgi.reshape(NCORES, NT, P).transpose(0, 2, 1))

    # S tiles: [P, NT, GPC] graph membership (bf16), node-major partitions
    S = np.zeros((NCORES, NT, P, GPC), np.float32)
    cnt_g = np.zeros((NCORES, GPC), np.float32)
    for c in range(NCORES):
        pm = perm[c]
        valid = pm >= 0
        g = graph_ids[pm[valid]] - c * GPC
        tt = np.arange(NPAD)[valid] // P
        pp = np.arange(NPAD)[valid] % P
        S[c, tt, pp, g] = 1.0
        np.add.at(cnt_g[c], g, 1.0)
    Sb = np.ascontiguousarray(S.transpose(0, 2, 1, 3)).astype(bf16)

    # reduce plan per tile: (table, k, token offset) for each live table
    tile_tabs = []
    for t in range(NT):
        entry = [(T, int(kg[t, 0, T]), int(tok_off[T, t, 0]))
                 for T in range(3) if kg[t, 0, T] > 0]
        tile_tabs.append(entry)
    live = [len(tile_tabs[t]) > 0 for t in range(NT)]

    # compact one-hot: 4 proteins per [128, 1002] tile, protein j at rows
    # [32j, 32j+25), ones (bias) row at 32j+25, zero guard cols 0/1001
    seq = np.asarray(inputs["protein_seq"]).reshape(NCORES, PPC // 4, 4, L)
    iot = np.arange(VOCAB)[None, None, None, :, None]
    one = np.float32(1)
    oh4 = np.zeros((NCORES, PPC // 4, 4, 32, L + 2), bf16)
    oh4[:, :, :, 0:VOCAB, 1:1 + L] = (seq[:, :, :, None, :] == iot) * one
    oh4[:, :, :, VOCAB, :] = one
    ohS = np.ascontiguousarray(
        oh4.reshape(NCORES, PPC // 4, 128, L + 2))

    # weights
    f32 = np.float32

    def b16(x):
        return np.ascontiguousarray(np.asarray(x, np.float32).astype(bf16))

    W_ri = np.asarray(inputs["W_ro_in"], f32)
    W_ro = np.asarray(inputs["W_ro_out"], f32)
    b_ri = np.asarray(inputs["b_ro_in"], f32)
    b_ro = np.asarray(inputs["b_ro_out"], f32)
    W_r2 = W_ri @ W_ro
    b_r2 = b_ri @ W_ro + b_ro                     # [HID]
    B2 = b_r2[None, :, None] * cnt_g[:, None, :]  # [NCORES, HID, GPC]

    # conv weights, tap-sliced lhsT with bias rows
    K1 = np.asarray(inputs["K1"], f32)            # [96, 128, 3]
    K2 = np.asarray(inputs["K2"], f32)            # [128, 96, 3]
    K3 = np.asarray(inputs["K3"], f32)            # [74, 128, 3]
    K4 = np.asarray(inputs["K4"], f32)            # [128, 74, 3]
    KT2e = np.zeros((97, 3, 128), f32)
    KT2e[:96] = K2.transpose(1, 2, 0)
    KT2e[96, 0, :] = np.asarray(inputs["cb2"], f32)
    KT3 = K3.transpose(1, 2, 0).copy()            # [128, 3, 74]
    KT4e = np.zeros((75, 3, 128), f32)
    KT4e[:74] = K4.transpose(1, 2, 0)
    KT4e[74, 0, :] = np.asarray(inputs["cb4"], f32)

    shared = {
        "tab0": tabs[0], "tab1": tabs[1], "tab2": tabs[2],
        "embT": b16(np.asarray(inputs["embed"], f32).T),      # [HID, 25]
        "K1T": b16(K1.transpose(1, 2, 0)),                    # [HID, 3, 96]
        "cb1row": b16(np.asarray(inputs["cb1"], f32).reshape(1, 96)),
        "KT2e": b16(KT2e), "KT3": b16(KT3), "KT4e": b16(KT4e),
        "cb3": np.asarray(inputs["cb3"], f32).reshape(IN_DIM, 1),
        "W_gc": b16(np.asarray(inputs["W_gc"], f32)),         # [74, HID]
        "b_gc": np.asarray(inputs["b_gc"], f32).reshape(HID, 1),
        "W_r2": b16(W_r2),
        "Wc1": b16(np.asarray(inputs["Wc1"], f32)),
        "bc1": np.asarray(inputs["bc1"], f32).reshape(HID, 1),
        "Wc2": b16(np.asarray(inputs["Wc2"], f32)),
        "bc2": np.asarray(inputs["bc2"], f32).reshape(HID, 1),
        "Wf1_r": b16(np.asarray(inputs["Wf1"], f32).reshape(2, HID, 2 * HID)
                     .transpose(1, 0, 2)),                    # [HID, 2, 256]
        "bf1_r": np.ascontiguousarray(
            np.asarray(inputs["bf1"], f32).reshape(2, HID, 1)
            .transpose(1, 0, 2)),                             # [HID, 2, 1]
        "Wf2_r": b16(np.asarray(inputs["Wf2"], f32).reshape(2, HID, 1)
                     .transpose(1, 0, 2)),                    # [HID, 2, 1]
        "bf2": np.asarray(inputs["bf2"], f32).reshape(1, 1),
        "ones2": np.ones((1, 1002), bf16),
    }
    percore = []
    for c in range(NCORES):
        percore.append({
            "ix0": idx_wrapped[0][c],
            "ix1": idx_wrapped[1][c],
            "ix2": idx_wrapped[2][c],
            "rdgi": np.ascontiguousarray(rdgi_pt[c]),
            "S": np.ascontiguousarray(Sb[c]),
            "ohS": np.ascontiguousarray(ohS[c]),
            "B2": np.ascontiguousarray(B2[c]),
        })
    meta = dict(NT=NT, sched=sched, tok_total=tok_total,
                tile_tabs=tile_tabs, live=live)
    return shared, percore, meta


# --------------------------------------------------------------- device build
def _build(shared, meta):
    NT = meta["NT"]
    sched = meta["sched"]
    tok_total = meta["tok_total"]
    tile_tabs = meta["tile_tabs"]
    live = meta["live"]

    nc = bacc.Bacc("TRN2", target_bir_lowering=False, debug=False,
                   num_devices=NCORES, num_swdge_queues=NQ)
    f32, bf, i16 = dt.float32, dt.bfloat16, dt.int16

    D = {k: nc.dram_tensor(k, list(v.shape), dt.from_np(v.dtype),
                           kind="ExternalInput")
         for k, v in shared.items()}
    for T in range(3):
        D["ix%d" % T] = nc.dram_tensor("ix%d" % T, [P, tok_total[T] // 16],
                                       i16, kind="ExternalInput")
    D["rdgi"] = nc.dram_tensor("rdgi", [P, NT], f32, kind="ExternalInput")
    D["S"] = nc.dram_tensor("S", [P, NT, GPC], bf, kind="ExternalInput")
    D["ohS"] = nc.dram_tensor("ohS", [PPC // 4, 128, L + 2], bf,
                              kind="ExternalInput")
    D["B2"] = nc.dram_tensor("B2", [HID, GPC], f32, kind="ExternalInput")
    out_d = nc.dram_tensor("out", [1, GPC], f32, kind="ExternalOutput")
    tabs = [D["tab%d" % T] for T in range(3)]

    with tile.TileContext(nc) as tc, contextlib.ExitStack() as ctx:
        wp = ctx.enter_context(tc.tile_pool(name="wp", bufs=1))
        gpool = ctx.enter_context(tc.tile_pool(name="gpool", bufs=1))
        ohp = ctx.enter_context(tc.tile_pool(name="ohp", bufs=2))
        redp = ctx.enter_context(tc.tile_pool(name="redp", bufs=4))
        aggp = ctx.enter_context(tc.tile_pool(name="aggp", bufs=1))
        gnp = ctx.enter_context(tc.tile_pool(name="gnp", bufs=3))
        pcv = ctx.enter_context(tc.tile_pool(name="pcv", bufs=4, space="PSUM"))
        pgn = ctx.enter_context(tc.tile_pool(name="pgn", bufs=2, space="PSUM"))
        ps1 = ctx.enter_context(tc.tile_pool(name="ps1", bufs=1, space="PSUM"))

        # ---------------- setup: weights/indices to SBUF
        def ld(name, shape, dtype, src=None, tag=None):
            t = wp.tile(shape, dtype, tag=tag or name)
            nc.sync.dma_start(out=t[:], in_=D[name][:] if src is None else src)
            return t

        ixs = [ld("ix%d" % T, [P, tok_total[T] // 16], i16) for T in range(3)]
        embT = ld("embT", [HID, VOCAB], bf)
        K1T = ld("K1T", [HID, 3, 96], bf)
        KT2e = ld("KT2e", [97, 3, 128], bf)
        KT3 = ld("KT3", [128, 3, IN_DIM], bf)
        KT4e = ld("KT4e", [75, 3, 128], bf)
        cb3 = ld("cb3", [IN_DIM, 1], f32)

        # M1r [128, 3, 96]: per-tap conv1 lhsT replicated at the four
        # 32-row offsets (matmul needs lhsT/rhs base partitions equal).
        # Rows 32j..32j+24 = embed @ K1_t^T, row 32j+25 = cb1 (tap 0 only),
        # rest zero (zeroed so NaN garbage cannot poison the psum).
        M1 = wp.tile([128, 3, 96], bf, tag="m1")
        nc.vector.memset(M1[:], 0.0)
        nc.sync.dma_start(out=M1[25:26, 0, :], in_=D["cb1row"][:])
        for t in range(3):
            pm = ps1.tile([VOCAB, 96], f32, space="PSUM", tag="ps1a")
            nc.tensor.matmul(pm[:], embT[:], K1T[:, t, :], start=True,
                             stop=True)
            nc.scalar.copy(M1[0:VOCAB, t, :], pm[:])
        for j in range(1, 4):
            nc.sync.dma_start(out=M1[32 * j:32 * j + 32, :, :],
                              in_=M1[0:32, :, :])

        # xs conv buffers: ones rows (matmul-folded bias) + zero guard cols
        x1 = wp.tile([97, 1002], bf, tag="xs1")
        x2 = wp.tile([128, 1002], bf, tag="xs2")
        x3 = wp.tile([75, 1002], bf, tag="xs3")
        nc.sync.dma_start(out=x1[96:97, :], in_=D["ones2"][:])
        nc.sync.dma_start(out=x3[74:75, :], in_=D["ones2"][:])
        for tl, nr in ((x1, 96), (x2, 128), (x3, 74)):
            nc.vector.memset(tl[0:nr, 0:1], 0.0)
            nc.vector.memset(tl[0:nr, 1001:1002], 0.0)

        chunkmax = wp.tile([HID, 2, PPC], f32, tag="chunkmax")
        ident = wp.tile([P, P], f32, tag="ident")
        make_identity(nc, ident[:])

        # ---------------- phase 1a: issue all gathers (gpsimd queues)
        gts = []
        for i, (T, off, ntok) in enumerate(sched):
            g = gpool.tile([P, ntok // P, P], bf, tag="g%d" % i)
            nc.gpsimd.dma_gather(
                out_ap=g[:], in_ap=tabs[T][:],
                idxs_ap=ixs[T][:, off // 16:(off + ntok) // 16],
                num_idxs=ntok, num_idxs_reg=ntok, elem_size=P,
                single_packet=False, queue_num=i % NQ)
            gts.append((T, off, ntok, g))

        def g_slice(T, toff, width):
            # locate the gather tile holding table-T tokens [toff, toff+width)
            for (Tg, off, ntok, g) in gts:
                if Tg == T and off <= toff and toff + width <= off + ntok:
                    b = (toff - off) // P
                    return g[:, b:b + width // P, 0:IN_DIM]
            raise AssertionError("token range not found")

        # ---------------- phase 1b: protein conv stack (dense)
        dmaq = [nc.sync, nc.scalar]
        for p in range(PPC):
            if p % 4 == 0:
                oh = ohp.tile([128, L + 2], bf, tag="oh%d" % ((p // 4) % 2))
                dmaq[(p // 4) % 2].dma_start(out=oh[:], in_=D["ohS"][p // 4])
            b0 = (p % 4) * 32
            # conv1: 3 taps, K=32 (one-hot block), bias via ones row
            pss1 = [pcv.tile([96, 500], f32, space="PSUM", tag="cps",
                             name="cps%d" % ci) for ci in range(2)]
            for tap in range(3):
                for ci, c0 in enumerate((0, 500)):
                    nc.tensor.matmul(
                        pss1[ci][:], M1[b0:b0 + 32, tap, :],
                        oh[b0:b0 + 32, c0 + tap:c0 + tap + 500],
                        start=(tap == 0), stop=(tap == 2),
                        tile_position=(96, 0) if b0 == 96 else None)
            for ci, c0 in enumerate((0, 500)):
                nc.scalar.activation(x1[0:96, 1 + c0:501 + c0], pss1[ci][:],
                                     AF.Relu)
            # conv2/3/4: 3 taps, 2 chunks, taps outer (weight reuse)
            for lyr, (KT, xin, nin) in enumerate(
                    ((KT2e, x1, 97), (KT3, x2, 128), (KT4e, x3, 75))):
                pss = [pcv.tile([CHANNELS[lyr + 2], 500], f32, space="PSUM",
                                tag="cps", name="cps%d" % ci)
                       for ci in range(2)]
                for tap in range(3):
                    for ci, c0 in enumerate((0, 500)):
                        nc.tensor.matmul(
                            pss[ci][:], KT[:, tap, :],
                            xin[0:nin, c0 + tap:c0 + tap + 500],
                            start=(tap == 0), stop=(tap == 2))
                for ci, c0 in enumerate((0, 500)):
                    if lyr == 0:    # -> xs2, relu via DVE (bias in psum)
                        nc.vector.tensor_scalar(
                            out=x2[:, 1 + c0:501 + c0], in0=pss[ci][:],
                            scalar1=0.0, scalar2=None, op0=ALU.max)
                    elif lyr == 1:  # -> xs3, relu+bias via ACT
                        nc.scalar.activation(
                            x3[0:IN_DIM, 1 + c0:501 + c0], pss[ci][:],
                            AF.Relu, bias=cb3[:])
                    else:           # conv4: max-pool the chunk
                        nc.vector.tensor_reduce(
                            out=chunkmax[:, ci, p:p + 1], in_=pss[ci][:],
                            axis=AX.X, op=ALU.max)

        # ---------------- phase 2: pmax, edge reduces, GNN chain
        # (wait-until hint orders all phase-2 ops after the conv stream in
        # every engine queue; runtime sync is still semaphore-driven)
        ctx.enter_context(tc.tile_wait_until(1.0))
        W_gc = ld("W_gc", [IN_DIM, HID], bf)
        b_gc = ld("b_gc", [HID, 1], f32)
        W_r2 = ld("W_r2", [HID, HID], bf)
        Wc1 = ld("Wc1", [HID, HID], bf); bc1 = ld("bc1", [HID, 1], f32)
        Wc2 = ld("Wc2", [HID, HID], bf); bc2 = ld("bc2", [HID, 1], f32)
        Wf1 = ld("Wf1_r", [HID, 2, 2 * HID], bf)
        bf1 = ld("bf1_r", [HID, 2, 1], f32)
        Wf2 = ld("Wf2_r", [HID, 2, 1], bf)
        bf2 = ld("bf2", [1, 1], f32)
        rdgi = ld("rdgi", [P, NT], f32)
        Sg = ld("S", [P, NT, GPC], bf)
        B2 = ld("B2", [HID, GPC], f32)
        mxt = wp.tile([HID, PPC], f32, tag="mxt")
        nc.vector.tensor_reduce(out=mxt[:],
                                in_=chunkmax[:].rearrange("p c q -> p q c"),
                                axis=AX.X, op=ALU.max)
        pmax = wp.tile([HID, PPC], bf, tag="pmax")
        nc.scalar.activation(pmax[:], mxt[:], AF.Relu)

        # per-tile segment sums: in-place bf16 tree-adds on the contiguous
        # gather slots, then f32 combine + scale, PE-transpose to bf16 aggT
        def g_block(T, toff):
            for (Tg, off, ntok, g) in gts:
                if Tg == T and off <= toff < off + ntok:
                    return g, (toff - off) // P
            raise AssertionError("token offset not found")

        aggs = []
        with nc.allow_low_precision(reason="bf16 edge sums, tol 2e-2"):
            for t in range(NT):
                agg = aggp.tile([IN_DIM, P], bf, tag="agg%d" % t)
                aggs.append(agg)
                if not live[t]:
                    nc.vector.memset(agg[:], 0.0)
                    continue
                slots = []
                for (T, k, toff) in tile_tabs[t]:
                    g, b = g_block(T, toff)
                    while k > 1:
                        h = k // 2
                        nc.vector.tensor_tensor(
                            out=g[:, b:b + h, :], in0=g[:, b:b + h, :],
                            in1=g[:, b + k - h:b + k, :], op=ALU.add)
                        k -= h
                    slots.append(g[:, b, :])
                if len(slots) == 3:   # fold the rare T2 slot in bf16
                    nc.vector.tensor_tensor(out=slots[0], in0=slots[0],
                                            in1=slots[2], op=ALU.add)
                    slots = slots[:2]
                acc = redp.tile([P, P], f32, tag="red0", name="red")
                if len(slots) == 2:
                    nc.vector.tensor_tensor(out=acc[:], in0=slots[0],
                                            in1=slots[1], op=ALU.add)
                else:
                    nc.vector.tensor_scalar(out=acc[:], in0=slots[0],
                                            scalar1=1.0, scalar2=None,
                                            op0=ALU.mult)
                nc.vector.tensor_scalar_mul(acc[:], acc[:], rdgi[:, t:t + 1])
                tp = pgn.tile([IN_DIM, P], f32, space="PSUM", tag="gps")
                nc.tensor.transpose(tp[:], acc[:, :IN_DIM], ident[:])
                nc.scalar.copy(agg[:], tp[:])

        # GNN chain per tile, accumulating hgT = sum_t x2n_t^T-free S product
        hgps = ps1.tile([HID, GPC], f32, space="PSUM", tag="hgps")
        lt = [t for t in range(NT)]
        for t in lt:
            hps = pgn.tile([HID, P], f32, space="PSUM", tag="gps")
            nc.tensor.matmul(hps[:], W_gc[:], aggs[t][:], start=True,
                             stop=True)
            h = gnp.tile([HID, P], bf, tag="h")
            nc.scalar.activation(h[:], hps[:], AF.Relu, bias=b_gc[:])
            x2ps = pgn.tile([P, HID], f32, space="PSUM", tag="gps")
            nc.tensor.matmul(x2ps[:], h[:], W_r2[:], start=True, stop=True)
            x2n = gnp.tile([P, HID], bf, tag="x2n")
            nc.scalar.copy(x2n[:], x2ps[:])
            nc.tensor.matmul(hgps[:], x2n[:], Sg[:, t, :],
                             start=(t == lt[0]), stop=(t == lt[-1]),
                             skip_group_check=True)
        hgf = wp.tile([HID, GPC], f32, tag="hgf")
        nc.vector.scalar_tensor_tensor(out=hgf[:], in0=hgps[:], scalar=1.0,
                                       in1=B2[:], op0=ALU.mult, op1=ALU.add)
        hg = wp.tile([HID, GPC], bf, tag="hg")
        nc.scalar.activation(hg[:], hgf[:], AF.Relu)
        # compound FC
        c1ps = pgn.tile([HID, GPC], f32, space="PSUM", tag="gps")
        nc.tensor.matmul(c1ps[:], Wc1[:], hg[:], start=True, stop=True)
        cv1 = wp.tile([HID, GPC], bf, tag="cv1")
        nc.scalar.activation(cv1[:], c1ps[:], AF.Relu, bias=bc1[:])
        c2ps = pgn.tile([HID, GPC], f32, space="PSUM", tag="gps")
        nc.tensor.matmul(c2ps[:], Wc2[:], cv1[:], start=True, stop=True)
        cv2 = wp.tile([HID, GPC], bf, tag="cv2")
        nc.scalar.activation(cv2[:], c2ps[:], AF.Relu, bias=bc2[:])
        # CPI head: z = [cv2; pmax]
        zin = [cv2, pmax]
        z2 = []
        for mc in range(2):
            zps = pgn.tile([HID, GPC], f32, space="PSUM", tag="gps")
            for kc in range(2):
                nc.tensor.matmul(zps[:], Wf1[:, kc, mc * HID:(mc + 1) * HID],
                                 zin[kc][:, :GPC], start=(kc == 0),
                                 stop=(kc == 1))
            zt = wp.tile([HID, GPC], bf, tag="z2_%d" % mc)
            nc.scalar.activation(zt[:], zps[:], AF.Relu, bias=bf1[:, mc, :])
            z2.append(zt)
        ops = ps1.tile([1, GPC], f32, space="PSUM", tag="ps1a")
        for kc in range(2):
            nc.tensor.matmul(ops[:], Wf2[:, kc, :], z2[kc][:],
                             start=(kc == 0), stop=(kc == 1))
        ot = wp.tile([1, GPC], f32, tag="ot")
        nc.scalar.activation(ot[:], ops[:], AF.Sigmoid, bias=bf2[:1, :])
        nc.sync.dma_start(out=out_d[:], in_=ot[:])

    nc.compile()
    return nc


def kernel(**inputs):
    shared, percore, meta = _host_prep(inputs)
    nc = _build(shared, meta)
    in_maps = []
    for c in range(NCORES):
        m = dict(shared)
        m.update(percore[c])
        in_maps.append(m)
    res = run_bass_kernel_spmd(nc, in_maps, list(range(NCORES)))
    out = np.concatenate([res.results[c]["out"].reshape(GPC)
                          for c in range(NCORES)])
    return out.reshape(B, 1).astype(np.float32)


if __name__ == "__main__":
    sys.path.insert(0, "/root/problem")
    import jax
    import reference
    with jax.default_device(jax.devices("cpu")[0]):
        inputs = {k: np.asarray(v) for k, v in reference.setup_inputs().items()}
        exp = np.asarray(reference.reference(**inputs))
    got = kernel(**inputs)
    err = np.abs(got - exp).max()
    rel = err / max(np.abs(exp).max(), 1e-9)
    print("max abs err:", err, " rel:", rel)
